# revision 1
# baseline (speedup 1.0000x reference)
"""DLSTMCell hypernetwork kernel for Trainium2 (runs on 4 of 8 NeuronCores).

Reference computation (per stock n of N=2048):
    mem  = emb_table[index]                       (N, 128)
    h1   = tanh(mem @ w1.T + b1)                  (N, 128)
    h    = tanh(h1 @ w2.T + b2)                   (N, 64)
    W_n  = (h @ w3.T + b3).reshape(192, 512)      per-stock LSTM weights
    z    = data_n @ W_n + lstm_bias               data = [x, hx]  (192,)
    g    = sigmoid(z); i,f,gg,o = split(g)
    cy   = cx*sigmoid(f) + sigmoid(i)*tanh(gg)
    hy   = sigmoid(o)*tanh(cy)

Key fusion: the (N, 192, 512) = 805MB weights tensor is never materialized.
    z[n,k] = sum_{d,b} (data[n,d]*h[n,b]) * W3perm[(d,b),k] + sum_d B3r[d,k]*data[n,d]
is a standard dense matmul with the SHARED (12288, 512) matrix W3perm against
per-stock outer-product tiles opT[(d,b), n], accumulated in PSUM.

Sharding: data-parallel over stocks on 4 cores (512 each). The 8 visible
cores oversubscribe the device: >4 concurrent cores measured ~2x slower
per core, so 4 cores give the best wall time. W3perm is replicated.

Precision: W3perm and the outer-product tiles are bf16 (measured end-to-end
rel err 6e-6 — indistinguishable from f32 here because the hypernetwork
matmul term is small against lstm_bias and two sigmoids compress errors);
everything else f32/f32r. PSUM accumulation is always f32.

Layout: gates kept transposed [k, n] so the gate unit k sits on partitions:
lstm_bias folds into the ACT sigmoid as a per-partition bias and the LSTM
epilogue runs on [128, n] tiles.

opT construction on-device, pipelined LA pair-units ahead of the gate
matmuls: A_t = rows (2t, 2t+1) of dataT each replicated 64x, built by a K=2
matmul against a constant 0/1 pattern (PE broadcast, writes PSUM);
opT = A * [hT; hT] on the vector engine, two K-tiles per DVE op.
"""
import sys

sys.path.insert(0, "/opt/trn_rl_repo")
import numpy as np
import ml_dtypes
import concourse.bacc as bacc
import concourse.mybir as mybir
import concourse.tile as tile
from concourse.bass_utils import run_bass_kernel_spmd

F32 = mybir.dt.float32
F32R = mybir.dt.float32r
BF16 = mybir.dt.bfloat16
FP8 = mybir.dt.float8e4
AF = mybir.ActivationFunctionType

USE_FP8 = False              # fp8e4 DoubleRow gate matmuls (2 K-tiles/matmul)
OP_SCALE = 8.0              # folded into the ppat broadcast constant
W3_SCALE = 64.0             # keeps w3 (~0.02 sigma) in e4m3 normal range
Z_DESCALE = 1.0 / (OP_SCALE * W3_SCALE)

N = 2048
INPUT = 64
EMB = 128
BOT = 64
HID = 128
WDIM = 4 * (INPUT + HID) * HID
NCORES = 4
NC_N = N // NCORES          # 512 stocks per core
D = INPUT + HID             # 192
K4 = 4 * HID                # 512 gate columns
KT = (D * BOT) // 128       # 96 contraction K-tiles of 128
KU = KT // 2                # 48 paired K-tiles

_cache = {}


def _build_program(repeat=1):
    """repeat>1 wraps the compute body in a hardware loop — used only for
    wall-clock slope timing (exec_ns ~= (wall[R2]-wall[R1])/(R2-R1))."""
    nc = bacc.Bacc(None)

    di = lambda name, shape, dt: nc.dram_tensor(name, shape, dt, kind="ExternalInput")
    memT_d = di("memT", [EMB, NC_N], F32R)
    dT0_d = di("dT0", [INPUT, NC_N], F32R)
    dT1_d = di("dT1", [HID, NC_N], F32R)
    # data row pairs packed 3-per-free-window at base partitions {0,32,64}:
    # pair t -> dPair3[32*(t%3) : +2, (t//3)*NC_N : (t//3+1)*NC_N].
    # DRAM carries only the 3 used row-pairs ([6, ...]); the zero rows of
    # the SBUF tile are never read so they need no DMA.
    dPair3_d = di("dPair3", [6, (KT // 3) * NC_N], F32R)
    cxT_d = di("cxT", [HID, NC_N], F32)
    w1T_d = di("w1T", [EMB, EMB], F32R)
    w2T_d = di("w2T", [EMB, BOT], F32R)
    b1_d = di("b1c", [EMB, 1], F32)
    b2_d = di("b2c", [BOT, 1], F32)
    # W3perm paired: rows 128u.. hold K-tiles 2u and 2u+1. bf16: side by
    # side [2u | 2u+1]; fp8 DoubleRow: interleaved [K, 2, k] with the pair
    # on the middle axis.
    w3p_d = di("w3p", [KU * 128, 2 * K4], FP8 if USE_FP8 else BF16)
    b3a_d = di("b3a", [INPUT, K4], F32R)
    b3b_d = di("b3b", [HID, K4], F32R)
    lb_d = di("lbias", [HID, 4], F32)
    ppat_d = di("ppat", [66, 128], F32R)
    hyT_o = nc.dram_tensor("hyT", [HID, NC_N], F32, kind="ExternalOutput")
    cyT_o = nc.dram_tensor("cyT", [HID, NC_N], F32, kind="ExternalOutput")

    with tile.TileContext(nc) as tc:
        with tc.tile_pool(name="const", bufs=1) as const, \
             tc.tile_pool(name="w3", bufs=6) as w3pool, \
             tc.tile_pool(name="op", bufs=4) as opool, \
             tc.tile_pool(name="ep", bufs=1) as ep, \
             tc.tile_pool(name="psA", bufs=2, space="PSUM") as psA, \
             tc.tile_pool(name="psG", bufs=1, space="PSUM") as psG:

            def load(dram, shape, dt, tag=None):
                nm = tag or dram.name
                t = const.tile(shape, dt, tag=nm, name=nm)
                nc.sync.dma_start(t[:], dram[:])
                return t

            memT = load(memT_d, [EMB, NC_N], F32R)
            dT0 = load(dT0_d, [INPUT, NC_N], F32R)
            dT1 = load(dT1_d, [HID, NC_N], F32R)
            dPair3 = const.tile([66, (KT // 3) * NC_N], F32R, name="dPair3")
            for gp in range(3):
                nc.sync.dma_start(
                    dPair3[32 * gp:32 * gp + 2, :], dPair3_d[2 * gp:2 * gp + 2, :]
                )
            cxT = load(cxT_d, [HID, NC_N], F32)
            w1T = load(w1T_d, [EMB, EMB], F32R)
            w2T = load(w2T_d, [EMB, BOT], F32R)
            b1c = load(b1_d, [EMB, 1], F32)
            b2c = load(b2_d, [BOT, 1], F32)
            b3a = load(b3a_d, [INPUT, K4], F32R)
            b3b = load(b3b_d, [HID, K4], F32R)
            lb = load(lb_d, [HID, 4], F32)
            ppat = load(ppat_d, [66, 128], F32R)

            from contextlib import ExitStack
            loop_ctx = ExitStack()
            if repeat > 1:
                loop_ctx.enter_context(
                    tc.For_i(0, repeat, 1, hint_engines=(mybir.EngineType.PE,))
                )

            # gate accumulators [k-chunk, n] — 4 full PSUM banks
            psg = [
                psG.tile([128, NC_N], F32, tag=f"g{kc}", name=f"psg{kc}")
                for kc in range(4)
            ]

            # hypernetwork MLP (PSUM scratch borrowed from psg banks; the
            # later start=True bias matmuls reset them for accumulation)
            nc.tensor.matmul(psg[0][:], w1T[:], memT[:], start=True, stop=True)
            h1T = ep.tile([128, NC_N], F32R, tag="h1T")
            nc.scalar.activation(h1T[:], psg[0][:], AF.Tanh, bias=b1c[:])
            nc.tensor.matmul(psg[1][0:BOT, :], w2T[:], h1T[:], start=True, stop=True)
            hT2 = ep.tile([128, NC_N], F32R, tag="hT2")
            nc.scalar.activation(hT2[0:BOT, :], psg[1][0:BOT, :], AF.Tanh, bias=b2c[:])
            nc.scalar.activation(hT2[BOT:128, :], psg[1][0:BOT, :], AF.Tanh, bias=b2c[:])

            # fold the b3 term in first (start=True resets the banks)
            for kc in range(4):
                ks = slice(kc * 128, kc * 128 + 128)
                nc.tensor.matmul(psg[kc][:], b3a[:, ks], dT0[:], start=True, stop=False)
                nc.tensor.matmul(psg[kc][:], b3b[:, ks], dT1[:], start=False, stop=False)

            # main contraction: 48 pair-units u = K-tiles (2u, 2u+1).
            # Stage A (per u): two K=2 broadcast matmuls -> pa2 [128,1024] PSUM,
            # one DVE mul -> op2 [128,2,512] bf16, one 512KB W3 DMA.
            # Gate matmuls consume pair u LA units later.
            LA = 2
            op_q = []
            w3_q = []

            def emit_stage_a(u):
                w3sb = w3pool.tile(
                    [128, 2, K4] if USE_FP8 else [128, 2 * K4],
                    FP8 if USE_FP8 else BF16, tag="w3sb", name="w3sb")
                src = w3p_d[u * 128:(u + 1) * 128, :]
                if USE_FP8:
                    src = src.rearrange("p (h k) -> p h k", h=2)
                nc.sync.dma_start(w3sb[:], src)
                w3_q.append(w3sb)
                pa2 = psA.tile([128, 2 * NC_N], F32, tag="A", name="pa2")
                for h in range(2):
                    t = 2 * u + h
                    gp, slot = t % 3, t // 3
                    nc.tensor.matmul(
                        pa2[:, h * NC_N:(h + 1) * NC_N],
                        ppat[32 * gp:32 * gp + 2, :],
                        dPair3[32 * gp:32 * gp + 2, slot * NC_N:(slot + 1) * NC_N],
                        start=True, stop=True,
                    )
                op2 = opool.tile([128, 2, NC_N], FP8 if USE_FP8 else BF16,
                                 tag="opT", name="op2")
                nc.vector.tensor_mul(
                    op2[:],
                    pa2[:].rearrange("p (h n) -> p h n", h=2),
                    hT2[:, None, :].broadcast_to([128, 2, NC_N]),
                )
                op_q.append(op2)

            for u in range(min(LA, KU)):
                emit_stage_a(u)
            for u in range(KU):
                if u + LA < KU:
                    emit_stage_a(u + LA)
                last = u == KU - 1
                if USE_FP8:
                    for kc in range(4):
                        nc.tensor.matmul(
                            psg[kc][:],
                            w3_q[u][:, :, kc * 128:kc * 128 + 128],
                            op_q[u][:],
                            start=False, stop=last,
                            perf_mode=mybir.MatmulPerfMode.DoubleRow,
                        )
                else:
                    for h in range(2):
                        for kc in range(4):
                            nc.tensor.matmul(
                                psg[kc][:],
                                w3_q[u][:, h * K4 + kc * 128:h * K4 + kc * 128 + 128],
                                op_q[u][:, h, :],
                                start=False, stop=last and h == 1,
                            )
                w3_q[u] = op_q[u] = None

            # LSTM epilogue on [hid, n] tiles; k-chunk order: i, f, g, o
            g = []
            for kc in range(4):
                gt = ep.tile([128, NC_N], F32, tag=f"gs{kc}", name=f"gs{kc}")
                nc.scalar.activation(gt[:], psg[kc][:], AF.Sigmoid,
                                     bias=lb[:, kc:kc + 1],
                                     scale=Z_DESCALE if USE_FP8 else 1.0)
                g.append(gt)
            i_t = ep.tile([128, NC_N], F32, tag="i_t")
            nc.scalar.activation(i_t[:], g[0][:], AF.Sigmoid)
            f_t = ep.tile([128, NC_N], F32, tag="f_t")
            nc.scalar.activation(f_t[:], g[1][:], AF.Sigmoid)
            g_t = ep.tile([128, NC_N], F32, tag="g_t")
            nc.scalar.activation(g_t[:], g[2][:], AF.Tanh)
            o_t = ep.tile([128, NC_N], F32, tag="o_t")
            nc.scalar.activation(o_t[:], g[3][:], AF.Sigmoid)

            t1 = ep.tile([128, NC_N], F32, tag="t1")
            nc.vector.tensor_mul(t1[:], cxT[:], f_t[:])
            t2 = ep.tile([128, NC_N], F32, tag="t2")
            nc.vector.tensor_mul(t2[:], i_t[:], g_t[:])
            cy = ep.tile([128, NC_N], F32, tag="cy")
            nc.vector.tensor_add(cy[:], t1[:], t2[:])
            tcy = ep.tile([128, NC_N], F32, tag="tcy")
            nc.scalar.activation(tcy[:], cy[:], AF.Tanh)
            hy = ep.tile([128, NC_N], F32, tag="hy")
            nc.vector.tensor_mul(hy[:], o_t[:], tcy[:])

            nc.sync.dma_start(cyT_o[:], cy[:])
            nc.sync.dma_start(hyT_o[:], hy[:])

            loop_ctx.close()

    nc.finalize()
    return nc


def kernel(x, index, hx, cx, emb_table, w1, b1, w2, b2, w3, b3, lstm_bias,
           _trace=False):
    x = np.asarray(x, np.float32)
    index = np.asarray(index)
    hx = np.asarray(hx, np.float32)
    cx = np.asarray(cx, np.float32)
    emb_table = np.asarray(emb_table, np.float32)
    w1 = np.asarray(w1, np.float32)
    b1 = np.asarray(b1, np.float32)
    w2 = np.asarray(w2, np.float32)
    b2 = np.asarray(b2, np.float32)
    w3 = np.asarray(w3, np.float32)
    b3 = np.asarray(b3, np.float32)
    lstm_bias = np.asarray(lstm_bias, np.float32)

    if "nc" not in _cache:
        _cache["nc"] = _build_program()
    nc = _cache["nc"]

    # host-side input prep (sharding + layout)
    mem = emb_table[index]                                   # (N, EMB)
    c = np.ascontiguousarray
    w1T = c(w1.T)
    w2T = c(w2.T)
    b1c = b1.reshape(EMB, 1)
    b2c = b2.reshape(BOT, 1)
    # W3perm[(d*64+b), k] = w3[d*512+k, b]; then pair K-tiles (2u, 2u+1)
    w3perm = w3.reshape(D, K4, BOT).transpose(0, 2, 1).reshape(D * BOT, K4)
    w3pair = w3perm.reshape(KU, 2, 128, K4).transpose(0, 2, 1, 3)
    if USE_FP8:
        w3p = c((w3pair * W3_SCALE).reshape(KU * 128, 2 * K4)).astype(
            ml_dtypes.float8_e4m3)
        zs = OP_SCALE * W3_SCALE
    else:
        w3p = c(w3pair.reshape(KU * 128, 2 * K4)).astype(ml_dtypes.bfloat16)
        zs = 1.0
    b3r = b3.reshape(D, K4) * zs
    b3a = c(b3r[0:INPUT])
    b3b = c(b3r[INPUT:D])
    lbias = c(lstm_bias.reshape(4, HID).T)                   # [j, kc]
    ppat = np.zeros((66, 128), np.float32)
    pv = OP_SCALE if USE_FP8 else 1.0
    for gp in range(3):
        ppat[32 * gp, 0:64] = pv
        ppat[32 * gp + 1, 64:128] = pv

    in_maps = []
    for ci in range(NCORES):
        sl = slice(ci * NC_N, (ci + 1) * NC_N)
        dataT = np.concatenate([x[sl].T, hx[sl].T], axis=0)  # (192, NC_N)
        dp3 = np.zeros((6, (KT // 3) * NC_N), np.float32)
        for t in range(KT):
            gp, slot = t % 3, t // 3
            dp3[2 * gp:2 * gp + 2, slot * NC_N:(slot + 1) * NC_N] = \
                dataT[2 * t:2 * t + 2]
        in_maps.append({
            "memT": c(mem[sl].T),
            "dT0": c(x[sl].T),
            "dT1": c(hx[sl].T),
            "dPair3": dp3,
            "cxT": c(cx[sl].T),
            "w1T": w1T, "w2T": w2T, "b1c": b1c, "b2c": b2c,
            "w3p": w3p, "b3a": b3a, "b3b": b3b,
            "lbias": lbias, "ppat": ppat,
        })

    res = run_bass_kernel_spmd(nc, in_maps, list(range(NCORES)), trace=_trace)
    hy = np.concatenate([r["hyT"].T for r in res.results], axis=0)
    cy = np.concatenate([r["cyT"].T for r in res.results], axis=0)
    if _trace:
        kernel.last_results = res
    return hy.astype(np.float32), cy.astype(np.float32)



# revision 2
# speedup vs baseline: 2.7686x; 2.7686x over previous
"""DLSTMCell hypernetwork kernel for Trainium2 (runs on 4 of 8 NeuronCores).

Reference computation (per stock n of N=2048):
    mem  = emb_table[index]                       (N, 128)
    h1   = tanh(mem @ w1.T + b1)                  (N, 128)
    h    = tanh(h1 @ w2.T + b2)                   (N, 64)
    W_n  = (h @ w3.T + b3).reshape(192, 512)      per-stock LSTM weights
    z    = data_n @ W_n + lstm_bias               data = [x, hx]  (192,)
    g    = sigmoid(z); i,f,gg,o = split(g)
    cy   = cx*sigmoid(f) + sigmoid(i)*tanh(gg)
    hy   = sigmoid(o)*tanh(cy)

Key fusion: the (N, 192, 512) = 805MB weights tensor is never materialized.
    z[n,k] = sum_{d,b} (data[n,d]*h[n,b]) * W3perm[(d,b),k] + sum_d B3r[d,k]*data[n,d]
is a standard dense matmul with the SHARED (12288, 512) matrix W3perm against
per-stock outer-product tiles opT[(d,b), n], accumulated in PSUM.

Sharding: data-parallel over stocks on 4 cores (512 each). The 8 visible
cores oversubscribe the device: >4 concurrent cores measured ~2x slower
per core, so 4 cores give the best wall time. W3perm is replicated.

Precision: W3perm and the outer-product tiles are bf16 (measured end-to-end
rel err 6e-6 — indistinguishable from f32 here because the hypernetwork
matmul term is small against lstm_bias and two sigmoids compress errors);
everything else f32/f32r. PSUM accumulation is always f32.

Layout: gates kept transposed [k, n] so the gate unit k sits on partitions:
lstm_bias folds into the ACT sigmoid as a per-partition bias and the LSTM
epilogue runs on [128, n] tiles.

opT construction on-device, pipelined LA pair-units ahead of the gate
matmuls: A_t = rows (2t, 2t+1) of dataT each replicated 64x, built by a K=2
matmul against a constant 0/1 pattern (PE broadcast, writes PSUM);
opT = A * [hT; hT] on the vector engine, two K-tiles per DVE op.
"""
import sys

sys.path.insert(0, "/opt/trn_rl_repo")
import numpy as np
import ml_dtypes
import concourse.bacc as bacc
import concourse.mybir as mybir
import concourse.tile as tile
from concourse.bass_utils import run_bass_kernel_spmd

F32 = mybir.dt.float32
F32R = mybir.dt.float32r
BF16 = mybir.dt.bfloat16
FP8 = mybir.dt.float8e4
AF = mybir.ActivationFunctionType

USE_FP8 = True              # fp8e4 DoubleRow gate matmuls (2 K-tiles/matmul)
OP_SCALE = 8.0              # folded into the ppat broadcast constant
W3_SCALE = 64.0             # keeps w3 (~0.02 sigma) in e4m3 normal range
Z_DESCALE = 1.0 / (OP_SCALE * W3_SCALE)

N = 2048
INPUT = 64
EMB = 128
BOT = 64
HID = 128
WDIM = 4 * (INPUT + HID) * HID
NCORES = 4
NC_N = N // NCORES          # 512 stocks per core
D = INPUT + HID             # 192
K4 = 4 * HID                # 512 gate columns
KT = (D * BOT) // 128       # 96 contraction K-tiles of 128
KU = KT // 2                # 48 paired K-tiles

_cache = {}


def _build_program(repeat=1):
    """repeat>1 wraps the compute body in a hardware loop — used only for
    wall-clock slope timing (exec_ns ~= (wall[R2]-wall[R1])/(R2-R1))."""
    nc = bacc.Bacc(None)

    di = lambda name, shape, dt: nc.dram_tensor(name, shape, dt, kind="ExternalInput")
    memT_d = di("memT", [EMB, NC_N], F32R)
    dT0_d = di("dT0", [INPUT, NC_N], F32R)
    dT1_d = di("dT1", [HID, NC_N], F32R)
    # data row pairs packed 3-per-free-window at base partitions {0,32,64}:
    # pair t -> dPair3[32*(t%3) : +2, (t//3)*NC_N : (t//3+1)*NC_N].
    # DRAM carries only the 3 used row-pairs ([6, ...]); the zero rows of
    # the SBUF tile are never read so they need no DMA.
    dPair3_d = di("dPair3", [6, (KT // 3) * NC_N], F32R)
    cxT_d = di("cxT", [HID, NC_N], F32)
    w1T_d = di("w1T", [EMB, EMB], F32R)
    w2T_d = di("w2T", [EMB, BOT], F32R)
    b1_d = di("b1c", [EMB, 1], F32)
    b2_d = di("b2c", [BOT, 1], F32)
    # W3perm paired: rows 128u.. hold K-tiles 2u and 2u+1. bf16: side by
    # side [2u | 2u+1]; fp8 DoubleRow: interleaved [K, 2, k] with the pair
    # on the middle axis.
    w3p_d = di("w3p", [KU * 128, 2 * K4], FP8 if USE_FP8 else BF16)
    b3a_d = di("b3a", [INPUT, K4], F32R)
    b3b_d = di("b3b", [HID, K4], F32R)
    lb_d = di("lbias", [HID, 4], F32)
    ppat_d = di("ppat", [66, 128], F32R)
    hyT_o = nc.dram_tensor("hyT", [HID, NC_N], F32, kind="ExternalOutput")
    cyT_o = nc.dram_tensor("cyT", [HID, NC_N], F32, kind="ExternalOutput")

    with tile.TileContext(nc) as tc:
        with tc.tile_pool(name="const", bufs=1) as const, \
             tc.tile_pool(name="w3", bufs=6) as w3pool, \
             tc.tile_pool(name="op", bufs=4) as opool, \
             tc.tile_pool(name="ep", bufs=1) as ep, \
             tc.tile_pool(name="psA", bufs=2, space="PSUM") as psA, \
             tc.tile_pool(name="psG", bufs=1, space="PSUM") as psG:

            def load(dram, shape, dt, tag=None):
                nm = tag or dram.name
                t = const.tile(shape, dt, tag=nm, name=nm)
                nc.sync.dma_start(t[:], dram[:])
                return t

            memT = load(memT_d, [EMB, NC_N], F32R)
            dT0 = load(dT0_d, [INPUT, NC_N], F32R)
            dT1 = load(dT1_d, [HID, NC_N], F32R)
            dPair3 = const.tile([66, (KT // 3) * NC_N], F32R, name="dPair3")
            for gp in range(3):
                nc.sync.dma_start(
                    dPair3[32 * gp:32 * gp + 2, :], dPair3_d[2 * gp:2 * gp + 2, :]
                )
            cxT = load(cxT_d, [HID, NC_N], F32)
            w1T = load(w1T_d, [EMB, EMB], F32R)
            w2T = load(w2T_d, [EMB, BOT], F32R)
            b1c = load(b1_d, [EMB, 1], F32)
            b2c = load(b2_d, [BOT, 1], F32)
            b3a = load(b3a_d, [INPUT, K4], F32R)
            b3b = load(b3b_d, [HID, K4], F32R)
            lb = load(lb_d, [HID, 4], F32)
            ppat = load(ppat_d, [66, 128], F32R)

            from contextlib import ExitStack
            loop_ctx = ExitStack()
            if repeat > 1:
                loop_ctx.enter_context(
                    tc.For_i(0, repeat, 1, hint_engines=(mybir.EngineType.PE,))
                )

            # gate accumulators [k-chunk, n] — 4 full PSUM banks
            psg = [
                psG.tile([128, NC_N], F32, tag=f"g{kc}", name=f"psg{kc}")
                for kc in range(4)
            ]

            # hypernetwork MLP (PSUM scratch borrowed from psg banks; the
            # later start=True bias matmuls reset them for accumulation)
            nc.tensor.matmul(psg[0][:], w1T[:], memT[:], start=True, stop=True)
            h1T = ep.tile([128, NC_N], F32R, tag="h1T")
            nc.scalar.activation(h1T[:], psg[0][:], AF.Tanh, bias=b1c[:])
            nc.tensor.matmul(psg[1][0:BOT, :], w2T[:], h1T[:], start=True, stop=True)
            hT2 = ep.tile([128, NC_N], F32R, tag="hT2")
            nc.scalar.activation(hT2[0:BOT, :], psg[1][0:BOT, :], AF.Tanh, bias=b2c[:])
            nc.scalar.activation(hT2[BOT:128, :], psg[1][0:BOT, :], AF.Tanh, bias=b2c[:])

            # fold the b3 term in first (start=True resets the banks)
            for kc in range(4):
                ks = slice(kc * 128, kc * 128 + 128)
                nc.tensor.matmul(psg[kc][:], b3a[:, ks], dT0[:], start=True, stop=False)
                nc.tensor.matmul(psg[kc][:], b3b[:, ks], dT1[:], start=False, stop=False)

            # main contraction: 48 pair-units u = K-tiles (2u, 2u+1).
            # Stage A (per u): two K=2 broadcast matmuls -> pa2 [128,1024] PSUM,
            # one DVE mul -> op2 [128,2,512] bf16, one 512KB W3 DMA.
            # Gate matmuls consume pair u LA units later.
            LA = 2
            op_q = []
            w3_q = []

            def emit_stage_a(u):
                w3sb = w3pool.tile(
                    [128, 2, K4] if USE_FP8 else [128, 2 * K4],
                    FP8 if USE_FP8 else BF16, tag="w3sb", name="w3sb")
                src = w3p_d[u * 128:(u + 1) * 128, :]
                if USE_FP8:
                    src = src.rearrange("p (h k) -> p h k", h=2)
                nc.sync.dma_start(w3sb[:], src)
                w3_q.append(w3sb)
                pa2 = psA.tile([128, 2 * NC_N], F32, tag="A", name="pa2")
                for h in range(2):
                    t = 2 * u + h
                    gp, slot = t % 3, t // 3
                    nc.tensor.matmul(
                        pa2[:, h * NC_N:(h + 1) * NC_N],
                        ppat[32 * gp:32 * gp + 2, :],
                        dPair3[32 * gp:32 * gp + 2, slot * NC_N:(slot + 1) * NC_N],
                        start=True, stop=True,
                    )
                op2 = opool.tile([128, 2, NC_N], FP8 if USE_FP8 else BF16,
                                 tag="opT", name="op2")
                nc.vector.tensor_mul(
                    op2[:],
                    pa2[:].rearrange("p (h n) -> p h n", h=2),
                    hT2[:, None, :].broadcast_to([128, 2, NC_N]),
                )
                op_q.append(op2)

            for u in range(min(LA, KU)):
                emit_stage_a(u)
            for u in range(KU):
                if u + LA < KU:
                    emit_stage_a(u + LA)
                last = u == KU - 1
                if USE_FP8:
                    for kc in range(4):
                        nc.tensor.matmul(
                            psg[kc][:],
                            w3_q[u][:, :, kc * 128:kc * 128 + 128],
                            op_q[u][:],
                            start=False, stop=last,
                            perf_mode=mybir.MatmulPerfMode.DoubleRow,
                        )
                else:
                    for h in range(2):
                        for kc in range(4):
                            nc.tensor.matmul(
                                psg[kc][:],
                                w3_q[u][:, h * K4 + kc * 128:h * K4 + kc * 128 + 128],
                                op_q[u][:, h, :],
                                start=False, stop=last and h == 1,
                            )
                w3_q[u] = op_q[u] = None

            # LSTM epilogue on [hid, n] tiles; k-chunk order: i, f, g, o
            g = []
            for kc in range(4):
                gt = ep.tile([128, NC_N], F32, tag=f"gs{kc}", name=f"gs{kc}")
                nc.scalar.activation(gt[:], psg[kc][:], AF.Sigmoid,
                                     bias=lb[:, kc:kc + 1],
                                     scale=Z_DESCALE if USE_FP8 else 1.0)
                g.append(gt)
            i_t = ep.tile([128, NC_N], F32, tag="i_t")
            nc.scalar.activation(i_t[:], g[0][:], AF.Sigmoid)
            f_t = ep.tile([128, NC_N], F32, tag="f_t")
            nc.scalar.activation(f_t[:], g[1][:], AF.Sigmoid)
            g_t = ep.tile([128, NC_N], F32, tag="g_t")
            nc.scalar.activation(g_t[:], g[2][:], AF.Tanh)
            o_t = ep.tile([128, NC_N], F32, tag="o_t")
            nc.scalar.activation(o_t[:], g[3][:], AF.Sigmoid)

            t1 = ep.tile([128, NC_N], F32, tag="t1")
            nc.vector.tensor_mul(t1[:], cxT[:], f_t[:])
            t2 = ep.tile([128, NC_N], F32, tag="t2")
            nc.vector.tensor_mul(t2[:], i_t[:], g_t[:])
            cy = ep.tile([128, NC_N], F32, tag="cy")
            nc.vector.tensor_add(cy[:], t1[:], t2[:])
            tcy = ep.tile([128, NC_N], F32, tag="tcy")
            nc.scalar.activation(tcy[:], cy[:], AF.Tanh)
            hy = ep.tile([128, NC_N], F32, tag="hy")
            nc.vector.tensor_mul(hy[:], o_t[:], tcy[:])

            nc.sync.dma_start(cyT_o[:], cy[:])
            nc.sync.dma_start(hyT_o[:], hy[:])

            loop_ctx.close()

    nc.finalize()
    return nc


def kernel(x, index, hx, cx, emb_table, w1, b1, w2, b2, w3, b3, lstm_bias,
           _trace=False):
    x = np.asarray(x, np.float32)
    index = np.asarray(index)
    hx = np.asarray(hx, np.float32)
    cx = np.asarray(cx, np.float32)
    emb_table = np.asarray(emb_table, np.float32)
    w1 = np.asarray(w1, np.float32)
    b1 = np.asarray(b1, np.float32)
    w2 = np.asarray(w2, np.float32)
    b2 = np.asarray(b2, np.float32)
    w3 = np.asarray(w3, np.float32)
    b3 = np.asarray(b3, np.float32)
    lstm_bias = np.asarray(lstm_bias, np.float32)

    if "nc" not in _cache:
        _cache["nc"] = _build_program()
    nc = _cache["nc"]

    # host-side input prep (sharding + layout)
    mem = emb_table[index]                                   # (N, EMB)
    c = np.ascontiguousarray
    w1T = c(w1.T)
    w2T = c(w2.T)
    b1c = b1.reshape(EMB, 1)
    b2c = b2.reshape(BOT, 1)
    # W3perm[(d*64+b), k] = w3[d*512+k, b]; then pair K-tiles (2u, 2u+1)
    w3perm = w3.reshape(D, K4, BOT).transpose(0, 2, 1).reshape(D * BOT, K4)
    w3pair = w3perm.reshape(KU, 2, 128, K4).transpose(0, 2, 1, 3)
    if USE_FP8:
        w3p = c((w3pair * W3_SCALE).reshape(KU * 128, 2 * K4)).astype(
            ml_dtypes.float8_e4m3)
        zs = OP_SCALE * W3_SCALE
    else:
        w3p = c(w3pair.reshape(KU * 128, 2 * K4)).astype(ml_dtypes.bfloat16)
        zs = 1.0
    b3r = b3.reshape(D, K4) * zs
    b3a = c(b3r[0:INPUT])
    b3b = c(b3r[INPUT:D])
    lbias = c(lstm_bias.reshape(4, HID).T)                   # [j, kc]
    ppat = np.zeros((66, 128), np.float32)
    pv = OP_SCALE if USE_FP8 else 1.0
    for gp in range(3):
        ppat[32 * gp, 0:64] = pv
        ppat[32 * gp + 1, 64:128] = pv

    in_maps = []
    for ci in range(NCORES):
        sl = slice(ci * NC_N, (ci + 1) * NC_N)
        dataT = np.concatenate([x[sl].T, hx[sl].T], axis=0)  # (192, NC_N)
        dp3 = np.zeros((6, (KT // 3) * NC_N), np.float32)
        for t in range(KT):
            gp, slot = t % 3, t // 3
            dp3[2 * gp:2 * gp + 2, slot * NC_N:(slot + 1) * NC_N] = \
                dataT[2 * t:2 * t + 2]
        in_maps.append({
            "memT": c(mem[sl].T),
            "dT0": c(x[sl].T),
            "dT1": c(hx[sl].T),
            "dPair3": dp3,
            "cxT": c(cx[sl].T),
            "w1T": w1T, "w2T": w2T, "b1c": b1c, "b2c": b2c,
            "w3p": w3p, "b3a": b3a, "b3b": b3b,
            "lbias": lbias, "ppat": ppat,
        })

    res = run_bass_kernel_spmd(nc, in_maps, list(range(NCORES)), trace=_trace)
    hy = np.concatenate([r["hyT"].T for r in res.results], axis=0)
    cy = np.concatenate([r["cyT"].T for r in res.results], axis=0)
    if _trace:
        kernel.last_results = res
    return hy.astype(np.float32), cy.astype(np.float32)



# revision 38
# speedup vs baseline: 5.8552x; 2.1148x over previous
"""DLSTMCell hypernetwork kernel for Trainium2 (runs on 4 of 8 NeuronCores).

Reference computation (per stock n of N=2048):
    mem  = emb_table[index]                       (N, 128)
    h1   = tanh(mem @ w1.T + b1)                  (N, 128)
    h    = tanh(h1 @ w2.T + b2)                   (N, 64)
    W_n  = (h @ w3.T + b3).reshape(192, 512)      per-stock LSTM weights
    z    = data_n @ W_n + lstm_bias               data = [x, hx]  (192,)
    g    = sigmoid(z); i,f,gg,o = split(g)
    cy   = cx*sigmoid(f) + sigmoid(i)*tanh(gg)
    hy   = sigmoid(o)*tanh(cy)

Key fusion: the (N, 192, 512) = 805MB weights tensor is never materialized.
    z[n,k] = sum_{d,b} (data[n,d]*h[n,b]) * W3perm[(d,b),k] + sum_d B3r[d,k]*data[n,d]
is a standard dense matmul with the SHARED (12288, 512) matrix W3perm against
per-stock outer-product tiles opT[(d,b), n], accumulated in PSUM.

Sharding: data-parallel over stocks on 4 cores (512 each). The 8 visible
cores oversubscribe the device: >4 concurrent cores measured ~2x slower
per core, so 4 cores give the best wall time. W3perm is replicated.

Precision: fp8e4 (e4m3) for W3perm and the outer-product tiles, with
DoubleRow gate matmuls (2 K-tiles per instruction); fp8 + DoublePixel for
the pa2 broadcast matmuls (f32r matmuls measured ~4x slower per row on
hardware than fp8/bf16); bf16 for the MLP and b3-fold matmuls. Numerics
are safe: the hypernetwork matmul term is ~1e-3 against lstm_bias and two
sigmoids compress errors (measured end-to-end rel err ~2e-4 vs the 2e-2
gate). PSUM accumulation is always f32.

Layout: gates kept transposed [k, n] so the gate unit k sits on partitions:
lstm_bias folds into the ACT sigmoid as a per-partition bias and the LSTM
epilogue runs on [128, n] tiles.

opT construction on-device, pipelined ahead of the gate matmuls with
decoupled lookaheads (w3 DMA LAW=5 units ahead, pa2/DVE LA=2 ahead):
A_t = rows (2t, 2t+1) of dataT each replicated 64x, built by a K=2
matmul against a constant 0/1 pattern (PE broadcast, writes PSUM);
opT = A * [hT; hT] on the vector engine, two K-tiles per DVE op.
"""
import sys

sys.path.insert(0, "/opt/trn_rl_repo")
import numpy as np
import ml_dtypes
import concourse.bacc as bacc
import concourse.mybir as mybir
import concourse.tile as tile
from concourse.bass_utils import run_bass_kernel_spmd

F32 = mybir.dt.float32
F32R = mybir.dt.float32r
BF16 = mybir.dt.bfloat16
FP8 = mybir.dt.float8e4
AF = mybir.ActivationFunctionType

USE_FP8 = True              # fp8e4 DoubleRow gate matmuls (2 K-tiles/matmul)
OP_SCALE = 8.0              # folded into the ppat broadcast constant
W3_SCALE = 64.0             # keeps w3 (~0.02 sigma) in e4m3 normal range
Z_DESCALE = 1.0 / (OP_SCALE * W3_SCALE)

N = 2048
INPUT = 64
EMB = 128
BOT = 64
HID = 128
WDIM = 4 * (INPUT + HID) * HID
NCORES = 4
D = INPUT + HID             # 192
K4 = 4 * HID                # 512 gate columns
KT = (D * BOT) // 128       # 96 contraction K-tiles of 128
KU = KT // 2                # 48 paired K-tiles

_cache = {}


def _build_program(repeat=1, ncores=NCORES, use_fp8=USE_FP8, gp_num=0,
                   gp_den=3, pa2_dp=True, aux_bf16=True, reorder=True,
                   debug_stage=None):
    """repeat>1 wraps the compute body in a hardware loop — used only for
    wall-clock slope timing (exec_ns ~= (wall[R2]-wall[R1])/(R2-R1)).
    gp_num/gp_den: fraction of op2 multiplies routed to the Pool engine
    (gpsimd) instead of DVE, to balance elementwise load.
    pa2_dp: run the pa2 broadcast matmuls in fp8 with DoublePixel."""
    nc_n = N // ncores
    nc = bacc.Bacc(None)

    AUX = BF16 if aux_bf16 else F32R
    di = lambda name, shape, dt: nc.dram_tensor(name, shape, dt, kind="ExternalInput")
    memT_d = di("memT", [EMB, nc_n], AUX)
    dT0_d = di("dT0", [INPUT, nc_n], AUX)
    dT1_d = di("dT1", [HID, nc_n], AUX)
    # data row pairs packed 3-per-free-window at base partitions {0,32,64}:
    # pair t -> dPair3[32*(t%3) : +2, (t//3)*nc_n : (t//3+1)*nc_n].
    # DRAM carries only the 3 used row-pairs ([6, ...]); the zero rows of
    # the SBUF tile are never read so they need no DMA.
    dPair3_d = di("dPair3", [6, (KT // 3) * nc_n], FP8 if pa2_dp else F32R)
    cxT_d = di("cxT", [HID, nc_n], F32)
    w1T_d = di("w1T", [EMB, EMB], AUX)
    w2T_d = di("w2T", [EMB, BOT], AUX)
    b1_d = di("b1c", [EMB, 1], F32)
    b2_d = di("b2c", [BOT, 1], F32)
    # W3perm paired: rows 128u.. hold K-tiles 2u and 2u+1. bf16: side by
    # side [2u | 2u+1]; fp8 DoubleRow: interleaved [K, 2, k] with the pair
    # on the middle axis.
    w3p_d = di("w3p", [KU * 128, 2 * K4], FP8 if use_fp8 else BF16)
    b3a_d = di("b3a", [INPUT, K4], AUX)
    b3b_d = di("b3b", [HID, K4], AUX)
    lb_d = di("lbias", [HID, 4], F32)
    ppat_d = di("ppat", [66, 128], FP8 if pa2_dp else F32R)
    hyT_o = nc.dram_tensor("hyT", [HID, nc_n], F32, kind="ExternalOutput")
    cyT_o = nc.dram_tensor("cyT", [HID, nc_n], F32, kind="ExternalOutput")

    with tile.TileContext(nc) as tc:
        with tc.tile_pool(name="const", bufs=1) as const, \
             tc.tile_pool(name="w3", bufs=6) as w3pool, \
             tc.tile_pool(name="op", bufs=4) as opool, \
             tc.tile_pool(name="ep", bufs=1) as ep, \
             tc.tile_pool(name="psA", bufs=2, space="PSUM") as psA, \
             tc.tile_pool(name="psG", bufs=1, space="PSUM") as psG:

            def load(dram, shape, dt, tag=None):
                nm = tag or dram.name
                t = const.tile(shape, dt, tag=nm, name=nm)
                nc.sync.dma_start(t[:], dram[:])
                return t

            memT = load(memT_d, [EMB, nc_n], AUX)
            dT0 = load(dT0_d, [INPUT, nc_n], AUX)
            dT1 = load(dT1_d, [HID, nc_n], AUX)
            dPair3 = const.tile([66, (KT // 3) * nc_n],
                                FP8 if pa2_dp else F32R, name="dPair3")
            for gp in range(3):
                nc.sync.dma_start(
                    dPair3[32 * gp:32 * gp + 2, :], dPair3_d[2 * gp:2 * gp + 2, :]
                )
            cxT = load(cxT_d, [HID, nc_n], F32)
            w1T = load(w1T_d, [EMB, EMB], AUX)
            w2T = load(w2T_d, [EMB, BOT], AUX)
            b1c = load(b1_d, [EMB, 1], F32)
            b2c = load(b2_d, [BOT, 1], F32)
            b3a = load(b3a_d, [INPUT, K4], AUX)
            b3b = load(b3b_d, [HID, K4], AUX)
            lb = load(lb_d, [HID, 4], F32)
            ppat = load(ppat_d, [66, 128], FP8 if pa2_dp else F32R)

            from contextlib import ExitStack
            loop_ctx = ExitStack()
            if repeat > 1:
                loop_ctx.enter_context(
                    tc.For_i(0, repeat, 1, hint_engines=(mybir.EngineType.PE,))
                )

            skip_stage_a = debug_stage is not None and debug_stage <= 1
            # gate accumulators [k-chunk, n] — 4 full PSUM banks (padded to
            # the 2KB bank so accumulation groups never share a bank)
            psg_pad = max(nc_n, 512)
            psg = [
                psG.tile([128, psg_pad], F32, tag=f"g{kc}", name=f"psg{kc}")
                [:, 0:nc_n]
                for kc in range(4)
            ]

            # stage-A plumbing (definitions used by the prefills below)
            LA = 2      # pa2/DVE lookahead (bounded by psA bufs)
            LAW = 5     # w3 DMA lookahead (bounded by w3pool bufs=6)
            op_q = []
            w3_q = []
            pa_q = []

            def emit_w3(u):
                w3sb = w3pool.tile(
                    [128, 2, K4] if use_fp8 else [128, 2 * K4],
                    FP8 if use_fp8 else BF16, tag="w3sb", name="w3sb")
                src = w3p_d[u * 128:(u + 1) * 128, :]
                if use_fp8:
                    src = src.rearrange("p (h k) -> p h k", h=2)
                nc.sync.dma_start(w3sb[:], src)
                w3_q.append(w3sb)

            def emit_pa_mm(u):
                pa2 = psA.tile([128, 2 * nc_n], F32, tag="A", name="pa2")
                for h in range(2):
                    t = 2 * u + h
                    gp, slot = t % 3, t // 3
                    nc.tensor.matmul(
                        pa2[:, h * nc_n:(h + 1) * nc_n],
                        ppat[32 * gp:32 * gp + 2, :],
                        dPair3[32 * gp:32 * gp + 2, slot * nc_n:(slot + 1) * nc_n],
                        start=True, stop=True,
                        perf_mode=mybir.MatmulPerfMode.DoublePixel
                        if pa2_dp else None,
                    )
                pa_q.append(pa2)

            def emit_pa_dve(u):
                op2 = opool.tile([128, 2, nc_n], FP8 if use_fp8 else BF16,
                                 tag="opT", name="op2")
                nc.vector.tensor_mul(
                    op2[:],
                    pa_q[u][:].rearrange("p (h n) -> p h n", h=2),
                    hT2[:, None, :].broadcast_to([128, 2, nc_n]),
                )
                pa_q[u] = None
                op_q.append(op2)

            # prefill ahead of the MLP: the PE's first work each iteration
            # depends only on DVE-freed psA bufs, never on the previous
            # iteration's epilogue ACT reads of the psg banks
            if not skip_stage_a and reorder:
                for u in range(min(LAW, KU)):
                    emit_w3(u)
                for u in range(min(LA, KU)):
                    emit_pa_mm(u)

            # hypernetwork MLP (PSUM scratch borrowed from psg banks; the
            # later start=True bias matmuls reset them for accumulation)
            nc.tensor.matmul(psg[0][:], w1T[:], memT[:], start=True, stop=True)
            h1T = ep.tile([128, nc_n], AUX, tag="h1T")
            nc.scalar.activation(h1T[:], psg[0][:], AF.Tanh, bias=b1c[:])
            nc.tensor.matmul(psg[1][0:BOT, :], w2T[:], h1T[:], start=True, stop=True)
            hT2 = ep.tile([128, nc_n], F32R, tag="hT2")
            nc.scalar.activation(hT2[0:BOT, :], psg[1][0:BOT, :], AF.Tanh, bias=b2c[:])
            nc.scalar.activation(hT2[BOT:128, :], psg[1][0:BOT, :], AF.Tanh, bias=b2c[:])

            # fold the b3 term in first (start=True resets the banks)
            for kc in range(4):
                ks = slice(kc * 128, kc * 128 + 128)
                nc.tensor.matmul(psg[kc][:], b3a[:, ks], dT0[:], start=True, stop=False)
                nc.tensor.matmul(psg[kc][:], b3b[:, ks], dT1[:], start=False,
                                 stop=skip_stage_a)

            # main contraction: 48 pair-units u = K-tiles (2u, 2u+1).
            # Stage A (per u): two K=2 broadcast matmuls -> pa2 [128,1024] PSUM,
            # one DVE mul -> op2 [128,2,512] fp8, one 512KB W3 DMA.
            # Gate matmuls consume pair u LA units later.
            if not skip_stage_a:
                if not reorder:
                    for u in range(min(LAW, KU)):
                        emit_w3(u)
                    for u in range(min(LA, KU)):
                        emit_pa_mm(u)
                for u in range(min(LA, KU)):
                    emit_pa_dve(u)
            for u in range(KU if not skip_stage_a else 0):
                if u + LAW < KU:
                    emit_w3(u + LAW)
                if u + LA < KU:
                    emit_pa_mm(u + LA)
                    emit_pa_dve(u + LA)
                last = u == KU - 1
                if use_fp8:
                    for kc in range(4):
                        nc.tensor.matmul(
                            psg[kc][:],
                            w3_q[u][:, :, kc * 128:kc * 128 + 128],
                            op_q[u][:],
                            start=False, stop=last,
                            perf_mode=mybir.MatmulPerfMode.DoubleRow,
                        )
                else:
                    for h in range(2):
                        for kc in range(4):
                            nc.tensor.matmul(
                                psg[kc][:],
                                w3_q[u][:, h * K4 + kc * 128:h * K4 + kc * 128 + 128],
                                op_q[u][:, h, :],
                                start=False, stop=last and h == 1,
                            )
                w3_q[u] = op_q[u] = None

            # LSTM epilogue on [hid, n] tiles; k-chunk order: i, f, g, o
            g = []
            for kc in range(4):
                gt = ep.tile([128, nc_n], F32, tag=f"gs{kc}", name=f"gs{kc}")
                nc.scalar.activation(gt[:], psg[kc][:], AF.Sigmoid,
                                     bias=lb[:, kc:kc + 1],
                                     scale=Z_DESCALE if use_fp8 else 1.0)
                g.append(gt)
            i_t = ep.tile([128, nc_n], F32, tag="i_t")
            nc.scalar.activation(i_t[:], g[0][:], AF.Sigmoid)
            f_t = ep.tile([128, nc_n], F32, tag="f_t")
            nc.scalar.activation(f_t[:], g[1][:], AF.Sigmoid)
            g_t = ep.tile([128, nc_n], F32, tag="g_t")
            nc.scalar.activation(g_t[:], g[2][:], AF.Tanh)
            o_t = ep.tile([128, nc_n], F32, tag="o_t")
            nc.scalar.activation(o_t[:], g[3][:], AF.Sigmoid)

            t1 = ep.tile([128, nc_n], F32, tag="t1")
            nc.vector.tensor_mul(t1[:], cxT[:], f_t[:])
            t2 = ep.tile([128, nc_n], F32, tag="t2")
            nc.vector.tensor_mul(t2[:], i_t[:], g_t[:])
            cy = ep.tile([128, nc_n], F32, tag="cy")
            nc.vector.tensor_add(cy[:], t1[:], t2[:])
            tcy = ep.tile([128, nc_n], F32, tag="tcy")
            nc.scalar.activation(tcy[:], cy[:], AF.Tanh)
            hy = ep.tile([128, nc_n], F32, tag="hy")
            nc.vector.tensor_mul(hy[:], o_t[:], tcy[:])

            nc.sync.dma_start(cyT_o[:], cy[:])
            nc.sync.dma_start(hyT_o[:], hy[:])

            loop_ctx.close()

    nc.finalize()
    return nc


def _prep_in_maps(x, index, hx, cx, emb_table, w1, b1, w2, b2, w3, b3,
                  lstm_bias, ncores=NCORES, use_fp8=USE_FP8, pa2_dp=True,
                  aux_bf16=True):
    """Host-side input prep (sharding + layout)."""
    nc_n = N // ncores
    aux = (lambda a: a.astype(ml_dtypes.bfloat16)) if aux_bf16 else \
        (lambda a: a)
    mem = emb_table[index]                                   # (N, EMB)
    c = np.ascontiguousarray
    w1T = aux(c(w1.T))
    w2T = aux(c(w2.T))
    b1c = b1.reshape(EMB, 1)
    b2c = b2.reshape(BOT, 1)
    # W3perm[(d*64+b), k] = w3[d*512+k, b]; then pair K-tiles (2u, 2u+1)
    w3perm = w3.reshape(D, K4, BOT).transpose(0, 2, 1).reshape(D * BOT, K4)
    w3pair = w3perm.reshape(KU, 2, 128, K4).transpose(0, 2, 1, 3)
    if use_fp8:
        w3p = c((w3pair * W3_SCALE).reshape(KU * 128, 2 * K4)).astype(
            ml_dtypes.float8_e4m3)
        zs = OP_SCALE * W3_SCALE
    else:
        w3p = c(w3pair.reshape(KU * 128, 2 * K4)).astype(ml_dtypes.bfloat16)
        zs = 1.0
    b3r = b3.reshape(D, K4) * zs
    b3a = aux(c(b3r[0:INPUT]))
    b3b = aux(c(b3r[INPUT:D]))
    lbias = c(lstm_bias.reshape(4, HID).T)                   # [j, kc]
    ppat = np.zeros((66, 128), np.float32)
    pv = OP_SCALE if use_fp8 else 1.0
    for gp in range(3):
        ppat[32 * gp, 0:64] = pv
        ppat[32 * gp + 1, 64:128] = pv
    if pa2_dp:
        ppat = ppat.astype(ml_dtypes.float8_e4m3)

    in_maps = []
    for ci in range(ncores):
        sl = slice(ci * nc_n, (ci + 1) * nc_n)
        dataT = np.concatenate([x[sl].T, hx[sl].T], axis=0)  # (192, nc_n)
        dp3 = np.zeros((6, (KT // 3) * nc_n), np.float32)
        for t in range(KT):
            gp, slot = t % 3, t // 3
            dp3[2 * gp:2 * gp + 2, slot * nc_n:(slot + 1) * nc_n] = \
                dataT[2 * t:2 * t + 2]
        if pa2_dp:
            dp3 = dp3.astype(ml_dtypes.float8_e4m3)
        in_maps.append({
            "memT": aux(c(mem[sl].T)),
            "dT0": aux(c(x[sl].T)),
            "dT1": aux(c(hx[sl].T)),
            "dPair3": dp3,
            "cxT": c(cx[sl].T),
            "w1T": w1T, "w2T": w2T, "b1c": b1c, "b2c": b2c,
            "w3p": w3p, "b3a": b3a, "b3b": b3b,
            "lbias": lbias, "ppat": ppat,
        })
    return in_maps


def kernel(x, index, hx, cx, emb_table, w1, b1, w2, b2, w3, b3, lstm_bias,
           _trace=False):
    x = np.asarray(x, np.float32)
    index = np.asarray(index)
    hx = np.asarray(hx, np.float32)
    cx = np.asarray(cx, np.float32)
    emb_table = np.asarray(emb_table, np.float32)
    w1 = np.asarray(w1, np.float32)
    b1 = np.asarray(b1, np.float32)
    w2 = np.asarray(w2, np.float32)
    b2 = np.asarray(b2, np.float32)
    w3 = np.asarray(w3, np.float32)
    b3 = np.asarray(b3, np.float32)
    lstm_bias = np.asarray(lstm_bias, np.float32)

    if "nc" not in _cache:
        _cache["nc"] = _build_program()
    nc = _cache["nc"]

    in_maps = _prep_in_maps(x, index, hx, cx, emb_table, w1, b1, w2, b2,
                            w3, b3, lstm_bias)
    res = run_bass_kernel_spmd(nc, in_maps, list(range(NCORES)), trace=_trace)
    hy = np.concatenate([r["hyT"].T for r in res.results], axis=0)
    cy = np.concatenate([r["cyT"].T for r in res.results], axis=0)
    if _trace:
        kernel.last_results = res
    return hy.astype(np.float32), cy.astype(np.float32)
